# revision 47
# baseline (speedup 1.0000x reference)
"""Trainium2 Bass kernel for nn_Grapher (GNN message passing block).

Strategy: pure data-parallel over batch B=64 -> 8 cores x 8 samples.
Per sample the edge conv collapses algebraically:
  max_k relu(BN(W_ec @ [x_i; x_j - x_i]))
    = relu(A[:,n] + max_k B[:,idx[n,k]] + shift)
with A = (W1-W2)*se @ h, B = W2*se @ h.  The KNN runs on a 210x210
cosine matrix via vector-engine max/max_index/match_replace.  The
9-neighbor gather of B rows goes through DRAM with InstDMAGatherAnt
(3 calls/sample, wrapped int16 indices replicated across the 8 Q7
cores); index order j = 256k + n lands row (n,k) at out[n%128,
2k + n//128, :] so the max tree runs on strided slices.  Mean-over-K
of the LoRA edge prompts commutes with the 1x1 conv and uses an
adjacency one-hot matmul.  BN scales/shifts are folded on the host.

Pipeline: 2 blocks x 2 pairs; stage A (fc1/lora/blend/norms) batched
per block so Gelu/Sqrt activation-table loads cluster; stage B skewed
(B1 = gram/top9/idx/AB/gathers, B2 = tree/transposes) to hide gather
DMA latency; stage C (fc2+ep) per pair.  fc1/lora/blend run in f32r
(1 PE pass); gram/norms stay f32 to keep the KNN ranking exact.
"""

import sys
from contextlib import ExitStack

import numpy as np

sys.path.insert(0, "/opt/trn_rl_repo")

import ml_dtypes  # noqa: E402
import concourse.bass as bass  # noqa: E402
import concourse.bacc as bacc  # noqa: E402
import concourse.mybir as mybir  # noqa: E402
import concourse.tile as tile  # noqa: E402
from concourse import library_config  # noqa: E402
from concourse.masks import make_identity  # noqa: E402

F32 = mybir.dt.float32
F32R = mybir.dt.float32r
USE_F32R = False
BF16 = mybir.dt.bfloat16
U32 = mybir.dt.uint32
I16 = mybir.dt.int16
AF = mybir.ActivationFunctionType
ALU = mybir.AluOpType

B, C, H, W = 64, 384, 14, 14
R, P, K = 32, 14, 9
H1, N = 15, 210
HW = H * W          # 196
EPS = 1e-5
NCORES = 8
SPC = B // NCORES   # samples per core = 8
NPAIRS = SPC // 2   # 4
CCH = C // 128      # 3 c-chunks
C2 = 2 * C          # 768
C2CH = C2 // 128    # 6
NT = (128, 82)      # node chunks: 210 = 128 + 82
NEG = -1.0e30
GELU_AF = AF.Gelu

_CACHE = {}


def _maybe_r(ap):
    return ap.bitcast(F32R) if USE_F32R else ap


def _build_nc():
    nc = bacc.Bacc(
        "TRN2", target_bir_lowering=False, debug=False,
        enable_asserts=False, num_devices=NCORES,
    )
    d = {}
    di = {
        "x_d": ([NPAIRS, 128, CCH, 2, HW], F32),
        "xb_d": ([NPAIRS, 128, CCH, 2, HW], BF16),
        "wfc1t": ([128, CCH, C], BF16),
        "bias1": ([128, CCH], F32),
        "prom": ([128, CCH, P], F32),
        "wdownt": ([128, CCH, R], BF16),
        "bdown": ([R, 1], F32),
        "gp": ([R, C], BF16),
        "wat": ([128, CCH, C2], BF16),
        "wbt": ([128, CCH, C2], BF16),
        "shifte": ([128, C2CH], F32),
        "wfc2t": ([128, C2CH, C], BF16),
        "wupt": ([R, C], BF16),
"shifto": ([128, CCH], F32),
    }
    for name, (shape, dt) in di.items():
        d[name] = nc.dram_tensor(name, shape, dt, kind="ExternalInput").ap()
    d["y_d"] = nc.dram_tensor(
        "y_d", [NPAIRS, 128, CCH, 2, HW], F32, kind="ExternalOutput"
    ).ap()
    return nc, d


def _build_program():
    nc, d = _build_nc()
    with tile.TileContext(nc) as tc:
        with ExitStack() as ctx:
            Emitter(ctx, tc, nc, d).emit()
    nc.compile()
    return nc


class Emitter:
    def __init__(self, ctx, tc, nc, d):
        self.ctx, self.tc, self.nc, self.d = ctx, tc, nc, d
        self.pc = {}   # per-pair tile context: pc[pair] = dict

    def emit(self):
        ctx, tc, nc, d = self.ctx, self.tc, self.nc, self.d
        nc.gpsimd.load_library(library_config.mlp)
        self.wp = ctx.enter_context(tc.tile_pool(name="weights", bufs=1))
        self.pa = ctx.enter_context(tc.tile_pool(name="pairp", bufs=1))
        self.hp_pool = ctx.enter_context(tc.tile_pool(name="hptr", bufs=1))
        self.sp = ctx.enter_context(tc.tile_pool(name="samp", bufs=3))
        self.gtp = ctx.enter_context(tc.tile_pool(name="gtp", bufs=4))
        self.pmm = ctx.enter_context(tc.tile_pool(name="pmm", bufs=2, space="PSUM"))
        self.pab = ctx.enter_context(tc.tile_pool(name="pab", bufs=2, space="PSUM"))
        self.dp = ctx.enter_context(tc.tile_pool(name="dscratch", bufs=4, space="DRAM"))

        wp = self.wp

        def wload(name, shape, dt):
            t = wp.tile(shape, dt, name=name)
            nc.sync.dma_start(t[:], d[name])
            return t

        self.wfc1t = wload("wfc1t", [128, CCH, C], BF16)
        self.bias1_early = True
        self.loads = {}
        self.bias1 = wload("bias1", [128, CCH], F32)
        for q, pr in enumerate((0, 1)):
            self.load_pair(pr, q)
        self.prom = wload("prom", [128, CCH, P], F32)
        self.wdownt = wload("wdownt", [128, CCH, R], BF16)
        self.bdown = wload("bdown", [R, 1], F32)
        self.gp = wload("gp", [R, C], BF16)
        self.wat = wload("wat", [128, CCH, C2], BF16)
        self.wbt = wload("wbt", [128, CCH, C2], BF16)
        self.shifte = wload("shifte", [128, C2CH], F32)
        self.wfc2t = wload("wfc2t", [128, C2CH, C], BF16)
        self.wupt = wload("wupt", [R, C], BF16)
        self.shifto = wload("shifto", [128, CCH], F32)

        self.z32 = wp.tile([48, K], U32, name="z32")
        nc.vector.memset(self.z32[:, :], 0)
        identf = wp.tile([128, 128], F32, name="identf")
        make_identity(nc, identf[:, :])
        self.identf = identf
        self.identb = wp.tile([128, 128], BF16, name="identb")
        nc.vector.tensor_copy(self.identb[:, :], identf[:, :])
        self.id08 = wp.tile([128, 128], BF16, name="id08")
        nc.vector.tensor_scalar_mul(self.id08[:, :], self.identb[:, :], 0.8)
        self.ones = wp.tile([128, 1], BF16, name="ones")
        nc.vector.memset(self.ones[:, :], 1.0)



        for q, pr in enumerate((2, 3)):
            self.load_pair(pr, q + 2)
        for pr in range(4):
            self.stage_a1(pr, pr)
        self.stage_a2(0, 0)
        self.stage_a2(1, 1)
        samples = [(pr, s2) for pr in range(4) for s2 in range(2)]
        for si, s in enumerate(samples):
            self.stage_b1(s)
            if si == 1:
                self.stage_a2(2, 2)
            elif si == 3:
                self.stage_a2(3, 3)
            if si >= 1:
                self.stage_b1b(samples[si - 1])
            if si >= 2:
                self.stage_b2(samples[si - 2])
        self.stage_b1b(samples[7])
        self.stage_b2(samples[6])
        self.stage_b2(samples[7])
        for pr in range(4):
            self.stage_c(pr, pr)

    def load_pair(self, pr, q):
        nc, d = self.nc, self.d
        xp = self.pa.tile([128, CCH, 2, HW], F32, tag=f"xp{q}")
        nc.sync.dma_start(xp[:], d["x_d"][pr])
        xb = self.pa.tile([128, CCH, 2, HW], BF16, tag=f"xb{q}")
        nc.sync.dma_start(xb[:], d["xb_d"][pr])
        self.loads[pr] = (xp, xb)

    # ---- stage A1: fc1 + lora(Gelu) + blend ----
    def stage_a1(self, pr, q):
        nc, d = self.nc, self.d
        pc = self.pc[pr] = {}
        xp, xb = self.loads[pr]
        pc["xp"] = xp

        hp = self.hp_pool.tile([128, CCH, 2, N], BF16, tag=f"hp{q}")
        pc["hp"] = hp
        for jo in range(CCH):
            ps = self.pmm.tile([128, 2, HW], F32, tag="mm")
            for ji in range(CCH):
                nc.tensor.matmul(
                    out=ps[:, :, :],
                    lhsT=self.wfc1t[:, ji, jo * 128:(jo + 1) * 128],
                    rhs=xb[:, ji, :, :],
                    start=(ji == 0), stop=(ji == CCH - 1),
                )
            nc.scalar.activation(
                hp[:, jo, :, :HW], ps[:, :, :], AF.Identity,
                bias=self.bias1[:, jo:jo + 1],
            )
        for s2 in range(2):
            nc.scalar.activation(hp[:, :, s2, HW:N], self.prom[:, :, :], AF.Copy)

        lrp = self.pa.tile([R, 2, N], BF16, tag=f"lrp{q}")
        psl = self.pmm.tile([R, 2, N], F32, tag="mm")
        for ji in range(CCH):
            nc.tensor.matmul(
                out=psl[:, :, :], lhsT=self.wdownt[:, ji, :],
                rhs=hp[:, ji, :, :],
                start=(ji == 0), stop=(ji == CCH - 1),
            )
        nc.scalar.activation(lrp[:, :, :], psl[:, :, :], GELU_AF,
                             bias=self.bdown[:, 0:1])
        pc["lrp"] = lrp
        pc["lrb"] = lrp

        hbb = self.pa.tile([128, CCH, 2, N], BF16, tag=f"hbb{q}")
        for jo in range(CCH):
            ps = self.pmm.tile([128, 2, N], F32, tag="mm")
            nc.tensor.matmul(out=ps[:, :, :],
                             lhsT=self.gp[:, jo * 128:(jo + 1) * 128],
                             rhs=lrp[:, :, :], start=True, stop=False)
            nc.tensor.matmul(out=ps[:, :, :], lhsT=self.id08[:, :],
                             rhs=hp[:, jo, :, :],
                             start=False, stop=True)
            nc.scalar.activation(hbb[:, jo, :, :], ps[:, :, :], AF.Copy)
        pc["hbp"] = hbb
        pc["hbb"] = hbb

        # reluT / lmp tiles persist until stage C
        reluT = self.pa.tile([128, C2CH, 2, N], BF16, tag=f"reluT{q}")
        lmp = self.pa.tile([R, 2, N], BF16, tag=f"lmp{q}")
        pc["reluT"], pc["lmp"] = reluT, lmp

    # ---- stage A2: column norms + cinv (Sqrt clustered per block) ----
    def stage_a2(self, pr, q):
        nc = self.nc
        pc = self.pc[pr]
        hbp = pc["hbp"]
        hsq = self.pc[pr]["hp"]
        nc.vector.tensor_mul(hsq[:, :, :, :], hbp[:, :, :, :], hbp[:, :, :, :])
        pss = self.pmm.tile([1, 2, N], F32, tag="mm")
        for ji in range(CCH):
            nc.tensor.matmul(out=pss[:, :, :], lhsT=self.ones[:, :],
                             rhs=hsq[:, ji, :, :],
                             start=(ji == 0), stop=(ji == CCH - 1))
        den = self.hp_pool.tile([1, 2, N], F32, tag=f"den{q}")
        nc.scalar.activation(den[:, :, :], pss[:, :, :], AF.Sqrt)
        nc.vector.tensor_scalar_add(den[:, :, :], den[:, :, :], 1e-12)
        cinv = self.pa.tile([1, 2, N], F32, tag=f"cinv{q}")
        nc.vector.reciprocal(cinv[:, :, :], den[:, :, :])
        pc["cinv"] = cinv
        cbc0 = self.pa.tile([128, N], F32, tag=f"cbc0{q}")
        nc.gpsimd.partition_broadcast(cbc0[:, :], cinv[:1, 0, :])
        cbc1 = self.pa.tile([128, N], F32, tag=f"cbc1{q}")
        nc.gpsimd.partition_broadcast(cbc1[:, :], cinv[:1, 1, :])
        pc["cbc"] = (cbc0, cbc1)

    # ---- stage B1: gram -> top9 -> idx chain -> A/B -> bvd -> gathers ----
    def stage_b1(self, s):
        pr, s2 = s
        nc, sp, dp = self.nc, self.sp, self.dp
        pc = self.pc[pr]
        hbp, hbb, cinv = pc["hbp"], pc["hbb"], pc["cinv"]

        cbc = pc["cbc"][s2]
        xn = sp.tile([128, CCH, N], BF16, tag="xn")
        for j in range(CCH):
            nc.vector.tensor_mul(xn[:, j, :], hbp[:, j, s2, :], cbc[:, :])

        # G[n, m] = hb[:,n] . xn[:,m]  (f32: KNN ranking accuracy)
        gs = sp.tile([128, 2, N], F32, tag="gs")
        for i, ni in enumerate(NT):
            ps = self.pmm.tile([128, N], F32, tag="mm")
            for j in range(CCH):
                nc.tensor.matmul(
                    out=ps[:ni, :],
                    lhsT=hbp[:, j, s2, i * 128:i * 128 + ni],
                    rhs=xn[:, j, :],
                    start=(j == 0), stop=(j == CCH - 1),
                )
            nc.scalar.activation(gs[:ni, i, :], ps[:ni, :], AF.Copy)

        # top-9 per row: top-8 (max/max_index) + 9th (match_replace)
        m8 = sp.tile([128, 2, 8], F32, tag="m8")
        i9 = sp.tile([128, 2, K], U32, tag="i9")
        gm = sp.tile([128, 2, N], F32, tag="gm")
        m8b = sp.tile([128, 2, 8], F32, tag="m8b")
        i8b = sp.tile([128, 2, 8], U32, tag="i8b")
        adj = sp.tile([128, 2, N], BF16, tag="adj")
        # wrapped int16 index list: j = 256k + n -> (n,k) at out[n%128, 2k+n//128]
        didx = dp.tile([256, K], U32, tag="didx")
        nc.scalar.dma_start(didx[208:256, :], self.z32[:, :])
        for i, ni in enumerate(NT):
            nc.vector.max(m8[:ni, i, :], gs[:ni, i, :])
            nc.vector.max_index(i9[:ni, i, 0:8], m8[:ni, i, :], gs[:ni, i, :])
            nc.vector.match_replace(gm[:ni, i, :], m8[:ni, i, :], gs[:ni, i, :], NEG)
            nc.vector.max(m8b[:ni, i, :], gm[:ni, i, :])
            nc.vector.max_index(i8b[:ni, i, :], m8b[:ni, i, :], gm[:ni, i, :])
            nc.vector.tensor_copy(i9[:ni, i, 8:9], i8b[:ni, i, 0:1])
            if i == 0:
                # launch the chunk-0 index write while chunk 1's top-9 runs
                nc.sync.dma_start(didx[0:128, :], i9[:, 0, :])
        nc.sync.dma_start(didx[128:210, :], i9[:82, 1, :])
        # adjacency one-hots are only needed by stage B2 -- keep them off the
        # top9 -> idx-chain critical path
        for i, ni in enumerate(NT):
            nc.vector.tensor_scalar(
                adj[:ni, i, :], gs[:ni, i, :], m8b[:ni, i, 0:1], None, op0=ALU.is_ge,
            )
        pc[("i9", s2)] = i9
        pc[("adj", s2)] = adj
        idxw = sp.tile([128, 3, 48], I16, tag="idxw")
        nc.vector.memset(idxw[:, :, :], 0)
        src = didx[:, :].bitcast(I16).rearrange(
            "(b p) (t a two) -> p t a b two", b=16, p=16, t=3, a=3, two=2)
        dst = idxw[0:16, :, :].rearrange("p t (a b) -> p t a b", a=3, b=16)
        nc.sync.dma_start(dst, src[:, :, :, :, 0])
        for g in (16, 32, 64):
            nc.sync.dma_start(idxw[g:2 * g, :, :], idxw[0:g, :, :])

        # A, B edge-conv halves (bf16), B -> DRAM for the gather
        ABp = sp.tile([128, 2, 2, C2], BF16, tag="ABp")
        bvd = dp.tile([N, C2], BF16, tag="bvd")
        for i, ni in enumerate(NT):
            for hf in range(2):
                ps = self.pab.tile([128, 2, 512], F32, tag="ab")
                for j in range(CCH):
                    lhs = hbb[:, j, s2, i * 128:i * 128 + ni]
                    nc.tensor.matmul(
                        out=ps[:ni, 0, 0:384], lhsT=lhs,
                        rhs=self.wat[:, j, hf * 384:(hf + 1) * 384],
                        start=(j == 0), stop=(j == CCH - 1),
                    )
                    nc.tensor.matmul(
                        out=ps[:ni, 1, 0:384], lhsT=lhs,
                        rhs=self.wbt[:, j, hf * 384:(hf + 1) * 384],
                        start=(j == 0), stop=(j == CCH - 1),
                    )
                nc.scalar.activation(
                    ABp[:ni, :, i, hf * 384:(hf + 1) * 384], ps[:ni, :, 0:384],
                    AF.Copy)
            nc.scalar.dma_start(bvd[i * 128:i * 128 + ni, :], ABp[:ni, 1, i, :])
        pc[("ABp", s2)] = ABp

        # issue the 3 gathers; the max tree runs one sample later (b1b)
        gts = []
        for t in range(3):
            nidx = 722 if t == 2 else 768
            ns = (nidx + 15) // 16
            gt = self.gtp.tile([128, 6, C2], BF16, tag="gt")
            nc.gpsimd.dma_gather(
                out_ap=gt[:, :, :], in_ap=bvd[:, :],
                idxs_ap=idxw[:, t, :ns], num_idxs=nidx, num_idxs_reg=nidx,
                elem_size=C2,
            )
            gts.append(gt)
        pc[("gts", s2)] = gts

    def stage_b1b(self, s):
        pr, s2 = s
        nc, sp = self.nc, self.sp
        pc = self.pc[pr]
        gts = pc[("gts", s2)]
        amax = sp.tile([128, 2, C2], BF16, tag="amax")
        for t, gt in enumerate(gts):
            if t == 0:
                nc.vector.tensor_tensor(out=amax[:, :, :], in0=gt[:, 0:2, :],
                                        in1=gt[:, 2:4, :], op=ALU.max)
                nc.vector.tensor_tensor(out=amax[:, :, :], in0=amax[:, :, :],
                                        in1=gt[:, 4:6, :], op=ALU.max)
            else:
                nc.vector.tensor_tensor(out=gt[:, 0:2, :], in0=gt[:, 0:2, :],
                                        in1=gt[:, 2:4, :], op=ALU.max)
                nc.vector.tensor_tensor(out=gt[:, 0:2, :], in0=gt[:, 0:2, :],
                                        in1=gt[:, 4:6, :], op=ALU.max)
                nc.vector.tensor_tensor(out=amax[:, :, :], in0=amax[:, :, :],
                                        in1=gt[:, 0:2, :], op=ALU.max)
        nc.vector.tensor_add(amax[:, :, :], pc[("ABp", s2)][:, 0, :, :],
                             amax[:, :, :])
        pc[("amax", s2)] = amax

    def stage_b2(self, s):
        pr, s2 = s
        nc, sp = self.nc, self.sp
        pc = self.pc[pr]
        amax, ABp = pc[("amax", s2)], pc[("ABp", s2)]
        i9, adj = pc[("i9", s2)], pc[("adj", s2)]
        reluT, lmp = pc["reluT"], pc["lmp"]
        lrb = pc["lrb"]

        am = amax

        # transpose am -> [c, n]; relu(+shift_e) -> reluT
        for cc in range(C2CH):
            pt = self.pab.tile([128, N], BF16, tag="tr")
            for i, ni in enumerate(NT):
                nc.tensor.transpose(
                    pt[:, i * 128:i * 128 + ni], am[:ni, i, cc * 128:(cc + 1) * 128],
                    self.identb[:ni, :ni])
            nc.scalar.activation(
                reluT[:, cc, s2, :], pt[:, :], AF.Relu,
                bias=self.shifte[:, cc:cc + 1],
            )

        # lr^T and Adj^T (bf16), lr_mean = (lr @ Adj^T)/9
        lrT = sp.tile([128, 2, R], BF16, tag="lrT")
        adjT = sp.tile([128, 2, N], BF16, tag="adjT")
        for i, ni in enumerate(NT):
            pt = self.pab.tile([128, N], BF16, tag="tr")
            nc.tensor.transpose(
                pt[:ni, :R], lrb[:, s2, i * 128:i * 128 + ni], self.identb[:R, :R])
            nc.scalar.activation(lrT[:ni, i, :], pt[:ni, :R], AF.Copy)
        for io, nio in enumerate(NT):
            pt = self.pab.tile([128, N], BF16, tag="tr")
            for ii, nii in enumerate(NT):
                nc.tensor.transpose(
                    pt[:nio, ii * 128:ii * 128 + nii],
                    adj[:nii, ii, io * 128:io * 128 + nio],
                    self.identb[:nii, :nii],
                )
            nc.scalar.activation(adjT[:nio, io, :], pt[:nio, :], AF.Copy)

        pslm = self.pmm.tile([R, N], F32, tag="mm")
        for i, ni in enumerate(NT):
            nc.tensor.matmul(
                out=pslm[:, :], lhsT=lrT[:ni, i, :], rhs=adjT[:ni, i, :],
                start=(i == 0), stop=(i == 1),
            )
        nc.scalar.activation(lmp[:, s2, :], pslm[:, :], AF.Copy, scale=1.0 / 9.0)

    # ---- stage C: fc2 + ep, residual, store ----
    def stage_c(self, pr, q):
        nc, d = self.nc, self.d
        pc = self.pc[pr]
        reluT, lmp, xp = pc["reluT"], pc["lmp"], pc["xp"]
        for jo in range(CCH):
            ps = self.pmm.tile([128, 2, N], F32, tag="mm")
            for jc in range(C2CH):
                nc.tensor.matmul(
                    out=ps[:, :, :], lhsT=self.wfc2t[:, jc, jo * 128:(jo + 1) * 128],
                    rhs=reluT[:, jc, :, :], start=(jc == 0), stop=False,
                )
            nc.tensor.matmul(out=ps[:, :, :],
                             lhsT=self.wupt[:, jo * 128:(jo + 1) * 128],
                             rhs=lmp[:, :, :], start=False, stop=True)
            tf = self.sp.tile([128, 2, HW], F32, tag="tf")
            nc.scalar.activation(tf[:, :, :], ps[:, :, :HW], AF.Identity,
                                 bias=self.shifto[:, jo:jo + 1])
            yo = self.sp.tile([128, 2, HW], F32, tag="yo")
            nc.vector.tensor_add(yo[:, :, :], tf[:, :, :], xp[:, jo, :, :])
            nc.sync.dma_start(d["y_d"][pr, :, jo, :, :], yo[:, :, :])


# ======================= host side =======================

def _prep_inputs(inputs):
    f32 = np.float32
    bf = ml_dtypes.bfloat16
    s1 = (inputs["bn1_g"] / np.sqrt(inputs["bn1_v"] + EPS)).astype(f32)
    Wfc1 = (inputs["w_fc1"] * s1[:, None]).astype(f32)
    b1 = ((inputs["b_fc1"] - inputs["bn1_m"]) * s1 + inputs["bn1_b"]).astype(f32)
    se = (inputs["bne_g"] / np.sqrt(inputs["bne_v"] + EPS)).astype(f32)
    W1 = inputs["w_ec"][:, :C]
    W2 = inputs["w_ec"][:, C:]
    WA = ((W1 - W2) * se[:, None]).astype(f32)
    WB = (W2 * se[:, None]).astype(f32)
    shift_e = ((inputs["b_ec"] - inputs["bne_m"]) * se + inputs["bne_b"]).astype(f32)
    s2 = (inputs["bn2_g"] / np.sqrt(inputs["bn2_v"] + EPS)).astype(f32)
    Wfc2 = (0.8 * inputs["w_fc2"] * s2[:, None]).astype(f32)
    wup = (0.2 * inputs["w_up"]).astype(f32)
    shift_out = (0.8 * ((inputs["b_fc2"] - inputs["bn2_m"]) * s2 + inputs["bn2_b"])
                 + 0.2 * inputs["b_up"]).astype(f32)

    def chunk_pj(a, nch):  # [nch*128, ...] -> [128, nch, ...]
        return np.ascontiguousarray(
            a.reshape(nch, 128, *a.shape[1:]).transpose(1, 0, *range(2, a.ndim + 1)))

    w = {
        "wfc1t": chunk_pj(Wfc1.T.copy(), CCH).astype(bf),       # [128,3,384]
        "bias1": chunk_pj(b1, CCH),                             # [128,3]
        "prom": chunk_pj(inputs["node_prompts"].astype(f32), CCH),
        "wdownt": chunk_pj(inputs["w_down"].T.astype(f32).copy(), CCH).astype(bf),
        "bdown": inputs["b_down"].astype(f32).reshape(R, 1),
        "gp": (0.2 * inputs["graph_prompt"]).astype(bf),        # [32,384]
        "wat": chunk_pj(WA.T.copy(), CCH).astype(bf),           # [128,3,768]
        "wbt": chunk_pj(WB.T.copy(), CCH).astype(bf),
        "shifte": chunk_pj(shift_e, C2CH),                      # [128,6]
        "wfc2t": chunk_pj(Wfc2.T.copy(), C2CH).astype(bf),      # [128,6,384]
        "wupt": wup.T.copy().astype(bf),                        # [32,384]
        "shifto": chunk_pj(shift_out, CCH),                     # [128,3]
    }
    w = {k: np.ascontiguousarray(v) for k, v in w.items()}
    return w


def _shard_x(x):
    # -> per-core [NPAIRS, 128, CCH, 2, HW] f32
    shards = []
    for c in range(NCORES):
        xs = x[c * SPC:(c + 1) * SPC].reshape(SPC, C, HW)
        xs = xs.reshape(NPAIRS, 2, CCH, 128, HW).transpose(0, 3, 2, 1, 4)
        shards.append(np.ascontiguousarray(xs.astype(np.float32)))
    return shards


def _unshard_y(results):
    out = np.empty((B, C, H, W), np.float32)
    for c in range(NCORES):
        y = results[c]["y_d"]  # [NPAIRS,128,CCH,2,HW]
        ys = y.transpose(0, 3, 2, 1, 4).reshape(SPC, C, H, W)
        out[c * SPC:(c + 1) * SPC] = ys
    return out


def get_program():
    if "nc" not in _CACHE:
        _CACHE["nc"] = _build_program()
    return _CACHE["nc"]


def run(inputs, trace=False, **kw):
    from concourse.bass_utils import run_bass_kernel_spmd
    nc = get_program()
    w = _prep_inputs(inputs)
    shards = _shard_x(np.asarray(inputs["x"], np.float32))
    import ml_dtypes as _md
    in_maps = [{**w, "x_d": shards[c],
                "xb_d": shards[c].astype(_md.bfloat16)} for c in range(NCORES)]
    res = run_bass_kernel_spmd(nc, in_maps, list(range(NCORES)), trace=trace, **kw)
    return _unshard_y(res.results), res


def kernel(**inputs):
    y, _ = run(inputs)
    return y


if __name__ == "__main__":
    get_program()
    print("program built OK")


# revision 49
# speedup vs baseline: 1.0404x; 1.0404x over previous
"""Trainium2 Bass kernel for nn_Grapher (GNN message passing block).

Strategy: pure data-parallel over batch B=64 -> 8 cores x 8 samples.
Per sample the edge conv collapses algebraically:
  max_k relu(BN(W_ec @ [x_i; x_j - x_i]))
    = relu(A[:,n] + max_k B[:,idx[n,k]] + shift)
with A = (W1-W2)*se @ h, B = W2*se @ h.  The KNN runs on a 210x210
cosine matrix via vector-engine max/max_index/match_replace.  The
9-neighbor gather of B rows goes through DRAM with InstDMAGatherAnt
(3 calls/sample, wrapped int16 indices replicated across the 8 Q7
cores); index order j = 256k + n lands row (n,k) at out[n%128,
2k + n//128, :] so the max tree runs on strided slices.  Mean-over-K
of the LoRA edge prompts commutes with the 1x1 conv and uses an
adjacency one-hot matmul.  BN scales/shifts are folded on the host.

Pipeline: 2 blocks x 2 pairs; stage A (fc1/lora/blend/norms) batched
per block so Gelu/Sqrt activation-table loads cluster; stage B skewed
(B1 = gram/top9/idx/AB/gathers, B2 = tree/transposes) to hide gather
DMA latency; stage C (fc2+ep) per pair.  fc1/lora/blend run in f32r
(1 PE pass); gram/norms stay f32 to keep the KNN ranking exact.
"""

import sys
from contextlib import ExitStack

import numpy as np

sys.path.insert(0, "/opt/trn_rl_repo")

import ml_dtypes  # noqa: E402
import concourse.bass as bass  # noqa: E402
import concourse.bacc as bacc  # noqa: E402
import concourse.mybir as mybir  # noqa: E402
import concourse.tile as tile  # noqa: E402
from concourse import library_config  # noqa: E402
from concourse.masks import make_identity  # noqa: E402

F32 = mybir.dt.float32
F32R = mybir.dt.float32r
USE_F32R = False
BF16 = mybir.dt.bfloat16
U32 = mybir.dt.uint32
I16 = mybir.dt.int16
AF = mybir.ActivationFunctionType
ALU = mybir.AluOpType

B, C, H, W = 64, 384, 14, 14
R, P, K = 32, 14, 9
H1, N = 15, 210
HW = H * W          # 196
EPS = 1e-5
NCORES = 8
SPC = B // NCORES   # samples per core = 8
NPAIRS = SPC // 2   # 4
CCH = C // 128      # 3 c-chunks
C2 = 2 * C          # 768
C2CH = C2 // 128    # 6
NT = (128, 82)      # node chunks: 210 = 128 + 82
NEG = -1.0e30
GELU_AF = AF.Gelu

_CACHE = {}


def _maybe_r(ap):
    return ap.bitcast(F32R) if USE_F32R else ap


def _build_nc():
    nc = bacc.Bacc(
        "TRN2", target_bir_lowering=False, debug=False,
        enable_asserts=False, num_devices=NCORES,
    )
    d = {}
    di = {
        "x_d": ([NPAIRS, 128, CCH, 2, HW], F32),
        "xb_d": ([NPAIRS, 128, CCH, 2, HW], BF16),
        "wfc1t": ([128, CCH, C], BF16),
        "bias1": ([128, CCH], F32),
        "prom": ([128, CCH, P], F32),
        "wdownt": ([128, CCH, R], BF16),
        "bdown": ([R, 1], F32),
        "gp": ([R, C], BF16),
        "wat": ([128, CCH, C2], BF16),
        "wbt": ([128, CCH, C2], BF16),
        "shifte": ([128, C2CH], F32),
        "wfc2t": ([128, C2CH, C], BF16),
        "wupt": ([R, C], BF16),
"shifto": ([128, CCH], F32),
    }
    for name, (shape, dt) in di.items():
        d[name] = nc.dram_tensor(name, shape, dt, kind="ExternalInput").ap()
    d["y_d"] = nc.dram_tensor(
        "y_d", [NPAIRS, 128, CCH, 2, HW], F32, kind="ExternalOutput"
    ).ap()
    return nc, d


def _build_program():
    nc, d = _build_nc()
    with tile.TileContext(nc) as tc:
        with ExitStack() as ctx:
            Emitter(ctx, tc, nc, d).emit()
    nc.compile()
    return nc


class Emitter:
    def __init__(self, ctx, tc, nc, d):
        self.ctx, self.tc, self.nc, self.d = ctx, tc, nc, d
        self.pc = {}   # per-pair tile context: pc[pair] = dict

    def emit(self):
        ctx, tc, nc, d = self.ctx, self.tc, self.nc, self.d
        nc.gpsimd.load_library(library_config.mlp)
        self.wp = ctx.enter_context(tc.tile_pool(name="weights", bufs=1))
        self.pa = ctx.enter_context(tc.tile_pool(name="pairp", bufs=1))
        self.hp_pool = ctx.enter_context(tc.tile_pool(name="hptr", bufs=1))
        self.sp = ctx.enter_context(tc.tile_pool(name="samp", bufs=3))
        self.gtp = ctx.enter_context(tc.tile_pool(name="gtp", bufs=4))
        self.pmm = ctx.enter_context(tc.tile_pool(name="pmm", bufs=2, space="PSUM"))
        self.pab = ctx.enter_context(tc.tile_pool(name="pab", bufs=2, space="PSUM"))
        self.dp = ctx.enter_context(tc.tile_pool(name="dscratch", bufs=4, space="DRAM"))

        wp = self.wp

        def wload(name, shape, dt):
            t = wp.tile(shape, dt, name=name)
            nc.sync.dma_start(t[:], d[name])
            return t

        self.wfc1t = wload("wfc1t", [128, CCH, C], BF16)
        self.bias1_early = True
        self.loads = {}
        self.bias1 = wload("bias1", [128, CCH], F32)
        for q, pr in enumerate((0, 1)):
            self.load_pair(pr, q)
        self.prom = wload("prom", [128, CCH, P], F32)
        self.wdownt = wload("wdownt", [128, CCH, R], BF16)
        self.bdown = wload("bdown", [R, 1], F32)
        self.gp = wload("gp", [R, C], BF16)
        self.wat = wload("wat", [128, CCH, C2], BF16)
        self.wbt = wload("wbt", [128, CCH, C2], BF16)
        self.shifte = wload("shifte", [128, C2CH], F32)
        self.wfc2t = wload("wfc2t", [128, C2CH, C], BF16)
        self.wupt = wload("wupt", [R, C], BF16)
        self.shifto = wload("shifto", [128, CCH], F32)

        self.z32 = wp.tile([48, K], U32, name="z32")
        nc.vector.memset(self.z32[:, :], 0)
        identf = wp.tile([128, 128], F32, name="identf")
        make_identity(nc, identf[:, :])
        self.identf = identf
        self.identb = wp.tile([128, 128], BF16, name="identb")
        nc.vector.tensor_copy(self.identb[:, :], identf[:, :])
        self.id08 = wp.tile([128, 128], BF16, name="id08")
        nc.vector.tensor_scalar_mul(self.id08[:, :], self.identb[:, :], 0.8)
        self.ones = wp.tile([128, 1], BF16, name="ones")
        nc.vector.memset(self.ones[:, :], 1.0)



        for q, pr in enumerate((2, 3)):
            self.load_pair(pr, q + 2)
        for pr in range(4):
            self.stage_a1(pr, pr)
        self.stage_a2(0, 0)
        self.stage_a2(1, 1)
        samples = [(pr, s2) for pr in range(4) for s2 in range(2)]
        for si, s in enumerate(samples):
            self.stage_b1(s)
            if si == 1:
                self.stage_a2(2, 2)
            elif si == 3:
                self.stage_a2(3, 3)
            if si >= 1:
                self.stage_b1b(samples[si - 1])
            if si >= 2:
                self.stage_b2(samples[si - 2])
            # emit fc2 for a pair as soon as both its B2 stages are done
            if si >= 3 and si % 2 == 1:
                pr_done = (si - 3) // 2
                self.stage_c(pr_done, pr_done)
        self.stage_b1b(samples[7])
        self.stage_b2(samples[6])
        self.stage_b2(samples[7])
        self.stage_c(3, 3)

    def load_pair(self, pr, q):
        nc, d = self.nc, self.d
        xp = self.pa.tile([128, CCH, 2, HW], F32, tag=f"xp{q}")
        nc.sync.dma_start(xp[:], d["x_d"][pr])
        xb = self.pa.tile([128, CCH, 2, HW], BF16, tag=f"xb{q}")
        nc.sync.dma_start(xb[:], d["xb_d"][pr])
        self.loads[pr] = (xp, xb)

    # ---- stage A1: fc1 + lora(Gelu) + blend ----
    def stage_a1(self, pr, q):
        nc, d = self.nc, self.d
        pc = self.pc[pr] = {}
        xp, xb = self.loads[pr]
        pc["xp"] = xp

        hp = self.hp_pool.tile([128, CCH, 2, N], BF16, tag=f"hp{q}")
        pc["hp"] = hp
        for jo in range(CCH):
            ps = self.pmm.tile([128, 2, HW], F32, tag="mm")
            for ji in range(CCH):
                nc.tensor.matmul(
                    out=ps[:, :, :],
                    lhsT=self.wfc1t[:, ji, jo * 128:(jo + 1) * 128],
                    rhs=xb[:, ji, :, :],
                    start=(ji == 0), stop=(ji == CCH - 1),
                )
            nc.scalar.activation(
                hp[:, jo, :, :HW], ps[:, :, :], AF.Identity,
                bias=self.bias1[:, jo:jo + 1],
            )
        for s2 in range(2):
            nc.scalar.activation(hp[:, :, s2, HW:N], self.prom[:, :, :], AF.Copy)

        lrp = self.pa.tile([R, 2, N], BF16, tag=f"lrp{q}")
        psl = self.pmm.tile([R, 2, N], F32, tag="mm")
        for ji in range(CCH):
            nc.tensor.matmul(
                out=psl[:, :, :], lhsT=self.wdownt[:, ji, :],
                rhs=hp[:, ji, :, :],
                start=(ji == 0), stop=(ji == CCH - 1),
            )
        nc.scalar.activation(lrp[:, :, :], psl[:, :, :], GELU_AF,
                             bias=self.bdown[:, 0:1])
        pc["lrp"] = lrp
        pc["lrb"] = lrp

        hbb = self.pa.tile([128, CCH, 2, N], BF16, tag=f"hbb{q}")
        for jo in range(CCH):
            ps = self.pmm.tile([128, 2, N], F32, tag="mm")
            nc.tensor.matmul(out=ps[:, :, :],
                             lhsT=self.gp[:, jo * 128:(jo + 1) * 128],
                             rhs=lrp[:, :, :], start=True, stop=False)
            nc.tensor.matmul(out=ps[:, :, :], lhsT=self.id08[:, :],
                             rhs=hp[:, jo, :, :],
                             start=False, stop=True)
            nc.scalar.activation(hbb[:, jo, :, :], ps[:, :, :], AF.Copy)
        pc["hbp"] = hbb
        pc["hbb"] = hbb

        # reluT / lmp tiles persist until stage C
        reluT = self.pa.tile([128, C2CH, 2, N], BF16, tag=f"reluT{q}")
        lmp = self.pa.tile([R, 2, N], BF16, tag=f"lmp{q}")
        pc["reluT"], pc["lmp"] = reluT, lmp

    # ---- stage A2: column norms + cinv (Sqrt clustered per block) ----
    def stage_a2(self, pr, q):
        nc = self.nc
        pc = self.pc[pr]
        hbp = pc["hbp"]
        hsq = self.pc[pr]["hp"]
        nc.vector.tensor_mul(hsq[:, :, :, :], hbp[:, :, :, :], hbp[:, :, :, :])
        pss = self.pmm.tile([1, 2, N], F32, tag="mm")
        for ji in range(CCH):
            nc.tensor.matmul(out=pss[:, :, :], lhsT=self.ones[:, :],
                             rhs=hsq[:, ji, :, :],
                             start=(ji == 0), stop=(ji == CCH - 1))
        den = self.hp_pool.tile([1, 2, N], F32, tag=f"den{q}")
        nc.scalar.activation(den[:, :, :], pss[:, :, :], AF.Sqrt)
        nc.vector.tensor_scalar_add(den[:, :, :], den[:, :, :], 1e-12)
        cinv = self.pa.tile([1, 2, N], F32, tag=f"cinv{q}")
        nc.vector.reciprocal(cinv[:, :, :], den[:, :, :])
        pc["cinv"] = cinv
        cbc0 = self.pa.tile([128, N], F32, tag=f"cbc0{q}")
        nc.gpsimd.partition_broadcast(cbc0[:, :], cinv[:1, 0, :])
        cbc1 = self.pa.tile([128, N], F32, tag=f"cbc1{q}")
        nc.gpsimd.partition_broadcast(cbc1[:, :], cinv[:1, 1, :])
        pc["cbc"] = (cbc0, cbc1)

    # ---- stage B1: gram -> top9 -> idx chain -> A/B -> bvd -> gathers ----
    def stage_b1(self, s):
        pr, s2 = s
        nc, sp, dp = self.nc, self.sp, self.dp
        pc = self.pc[pr]
        hbp, hbb, cinv = pc["hbp"], pc["hbb"], pc["cinv"]

        cbc = pc["cbc"][s2]
        xn = sp.tile([128, CCH, N], BF16, tag="xn")
        for j in range(CCH):
            nc.vector.tensor_mul(xn[:, j, :], hbp[:, j, s2, :], cbc[:, :])

        # G[n, m] = hb[:,n] . xn[:,m]  (f32: KNN ranking accuracy)
        gs = sp.tile([128, 2, N], F32, tag="gs")
        for i, ni in enumerate(NT):
            ps = self.pmm.tile([128, N], F32, tag="mm")
            for j in range(CCH):
                nc.tensor.matmul(
                    out=ps[:ni, :],
                    lhsT=hbp[:, j, s2, i * 128:i * 128 + ni],
                    rhs=xn[:, j, :],
                    start=(j == 0), stop=(j == CCH - 1),
                )
            nc.scalar.activation(gs[:ni, i, :], ps[:ni, :], AF.Copy)

        # top-9 per row: top-8 (max/max_index) + 9th (match_replace)
        m8 = sp.tile([128, 2, 8], F32, tag="m8")
        i9 = sp.tile([128, 2, K], U32, tag="i9")
        gm = sp.tile([128, 2, N], F32, tag="gm")
        m8b = sp.tile([128, 2, 8], F32, tag="m8b")
        i8b = sp.tile([128, 2, 8], U32, tag="i8b")
        adj = sp.tile([128, 2, N], BF16, tag="adj")
        # wrapped int16 index list: j = 256k + n -> (n,k) at out[n%128, 2k+n//128]
        didx = dp.tile([256, K], U32, tag="didx")
        nc.scalar.dma_start(didx[208:256, :], self.z32[:, :])
        for i, ni in enumerate(NT):
            nc.vector.max(m8[:ni, i, :], gs[:ni, i, :])
            nc.vector.max_index(i9[:ni, i, 0:8], m8[:ni, i, :], gs[:ni, i, :])
            nc.vector.match_replace(gm[:ni, i, :], m8[:ni, i, :], gs[:ni, i, :], NEG)
            nc.vector.max(m8b[:ni, i, :], gm[:ni, i, :])
            nc.vector.max_index(i8b[:ni, i, :], m8b[:ni, i, :], gm[:ni, i, :])
            nc.vector.tensor_copy(i9[:ni, i, 8:9], i8b[:ni, i, 0:1])
            if i == 0:
                # launch the chunk-0 index write while chunk 1's top-9 runs
                nc.sync.dma_start(didx[0:128, :], i9[:, 0, :])
        nc.sync.dma_start(didx[128:210, :], i9[:82, 1, :])
        # adjacency one-hots are only needed by stage B2 -- keep them off the
        # top9 -> idx-chain critical path
        for i, ni in enumerate(NT):
            nc.vector.tensor_scalar(
                adj[:ni, i, :], gs[:ni, i, :], m8b[:ni, i, 0:1], None, op0=ALU.is_ge,
            )
        pc[("i9", s2)] = i9
        pc[("adj", s2)] = adj
        idxw = sp.tile([128, 3, 48], I16, tag="idxw")
        nc.vector.memset(idxw[:, :, :], 0)
        src = didx[:, :].bitcast(I16).rearrange(
            "(b p) (t a two) -> p t a b two", b=16, p=16, t=3, a=3, two=2)
        dst = idxw[0:16, :, :].rearrange("p t (a b) -> p t a b", a=3, b=16)
        nc.sync.dma_start(dst, src[:, :, :, :, 0])
        for g in (16, 32, 64):
            nc.sync.dma_start(idxw[g:2 * g, :, :], idxw[0:g, :, :])

        # A, B edge-conv halves (bf16), B -> DRAM for the gather
        ABp = sp.tile([128, 2, 2, C2], BF16, tag="ABp")
        bvd = dp.tile([N, C2], BF16, tag="bvd")
        for i, ni in enumerate(NT):
            for hf in range(2):
                ps = self.pab.tile([128, 2, 512], F32, tag="ab")
                for j in range(CCH):
                    lhs = hbb[:, j, s2, i * 128:i * 128 + ni]
                    nc.tensor.matmul(
                        out=ps[:ni, 0, 0:384], lhsT=lhs,
                        rhs=self.wat[:, j, hf * 384:(hf + 1) * 384],
                        start=(j == 0), stop=(j == CCH - 1),
                    )
                    nc.tensor.matmul(
                        out=ps[:ni, 1, 0:384], lhsT=lhs,
                        rhs=self.wbt[:, j, hf * 384:(hf + 1) * 384],
                        start=(j == 0), stop=(j == CCH - 1),
                    )
                nc.scalar.activation(
                    ABp[:ni, :, i, hf * 384:(hf + 1) * 384], ps[:ni, :, 0:384],
                    AF.Copy)
            nc.scalar.dma_start(bvd[i * 128:i * 128 + ni, :], ABp[:ni, 1, i, :])
        pc[("ABp", s2)] = ABp

        # issue the 3 gathers; the max tree runs one sample later (b1b)
        gts = []
        for t in range(3):
            nidx = 722 if t == 2 else 768
            ns = (nidx + 15) // 16
            gt = self.gtp.tile([128, 6, C2], BF16, tag="gt")
            nc.gpsimd.dma_gather(
                out_ap=gt[:, :, :], in_ap=bvd[:, :],
                idxs_ap=idxw[:, t, :ns], num_idxs=nidx, num_idxs_reg=nidx,
                elem_size=C2,
            )
            gts.append(gt)
        pc[("gts", s2)] = gts

    def stage_b1b(self, s):
        pr, s2 = s
        nc, sp = self.nc, self.sp
        pc = self.pc[pr]
        gts = pc[("gts", s2)]
        amax = sp.tile([128, 2, C2], BF16, tag="amax")
        for t, gt in enumerate(gts):
            if t == 0:
                nc.vector.tensor_tensor(out=amax[:, :, :], in0=gt[:, 0:2, :],
                                        in1=gt[:, 2:4, :], op=ALU.max)
                nc.vector.tensor_tensor(out=amax[:, :, :], in0=amax[:, :, :],
                                        in1=gt[:, 4:6, :], op=ALU.max)
            else:
                nc.vector.tensor_tensor(out=gt[:, 0:2, :], in0=gt[:, 0:2, :],
                                        in1=gt[:, 2:4, :], op=ALU.max)
                nc.vector.tensor_tensor(out=gt[:, 0:2, :], in0=gt[:, 0:2, :],
                                        in1=gt[:, 4:6, :], op=ALU.max)
                nc.vector.tensor_tensor(out=amax[:, :, :], in0=amax[:, :, :],
                                        in1=gt[:, 0:2, :], op=ALU.max)
        nc.vector.tensor_add(amax[:, :, :], pc[("ABp", s2)][:, 0, :, :],
                             amax[:, :, :])
        pc[("amax", s2)] = amax

    def stage_b2(self, s):
        pr, s2 = s
        nc, sp = self.nc, self.sp
        pc = self.pc[pr]
        amax, ABp = pc[("amax", s2)], pc[("ABp", s2)]
        i9, adj = pc[("i9", s2)], pc[("adj", s2)]
        reluT, lmp = pc["reluT"], pc["lmp"]
        lrb = pc["lrb"]

        am = amax

        # transpose am -> [c, n]; relu(+shift_e) -> reluT
        for cc in range(C2CH):
            pt = self.pab.tile([128, N], BF16, tag="tr")
            for i, ni in enumerate(NT):
                nc.tensor.transpose(
                    pt[:, i * 128:i * 128 + ni], am[:ni, i, cc * 128:(cc + 1) * 128],
                    self.identb[:ni, :ni])
            nc.scalar.activation(
                reluT[:, cc, s2, :], pt[:, :], AF.Relu,
                bias=self.shifte[:, cc:cc + 1],
            )

        # lr^T and Adj^T (bf16), lr_mean = (lr @ Adj^T)/9
        lrT = sp.tile([128, 2, R], BF16, tag="lrT")
        adjT = sp.tile([128, 2, N], BF16, tag="adjT")
        for i, ni in enumerate(NT):
            pt = self.pab.tile([128, N], BF16, tag="tr")
            nc.tensor.transpose(
                pt[:ni, :R], lrb[:, s2, i * 128:i * 128 + ni], self.identb[:R, :R])
            nc.scalar.activation(lrT[:ni, i, :], pt[:ni, :R], AF.Copy)
        for io, nio in enumerate(NT):
            pt = self.pab.tile([128, N], BF16, tag="tr")
            for ii, nii in enumerate(NT):
                nc.tensor.transpose(
                    pt[:nio, ii * 128:ii * 128 + nii],
                    adj[:nii, ii, io * 128:io * 128 + nio],
                    self.identb[:nii, :nii],
                )
            nc.scalar.activation(adjT[:nio, io, :], pt[:nio, :], AF.Copy)

        pslm = self.pmm.tile([R, N], F32, tag="mm")
        for i, ni in enumerate(NT):
            nc.tensor.matmul(
                out=pslm[:, :], lhsT=lrT[:ni, i, :], rhs=adjT[:ni, i, :],
                start=(i == 0), stop=(i == 1),
            )
        nc.scalar.activation(lmp[:, s2, :], pslm[:, :], AF.Copy, scale=1.0 / 9.0)

    # ---- stage C: fc2 + ep, residual, store ----
    def stage_c(self, pr, q):
        nc, d = self.nc, self.d
        pc = self.pc[pr]
        reluT, lmp, xp = pc["reluT"], pc["lmp"], pc["xp"]
        for jo in range(CCH):
            ps = self.pmm.tile([128, 2, N], F32, tag="mm")
            for jc in range(C2CH):
                nc.tensor.matmul(
                    out=ps[:, :, :], lhsT=self.wfc2t[:, jc, jo * 128:(jo + 1) * 128],
                    rhs=reluT[:, jc, :, :], start=(jc == 0), stop=False,
                )
            nc.tensor.matmul(out=ps[:, :, :],
                             lhsT=self.wupt[:, jo * 128:(jo + 1) * 128],
                             rhs=lmp[:, :, :], start=False, stop=True)
            tf = self.sp.tile([128, 2, HW], F32, tag="tf")
            nc.scalar.activation(tf[:, :, :], ps[:, :, :HW], AF.Identity,
                                 bias=self.shifto[:, jo:jo + 1])
            yo = self.sp.tile([128, 2, HW], F32, tag="yo")
            nc.vector.tensor_add(yo[:, :, :], tf[:, :, :], xp[:, jo, :, :])
            nc.sync.dma_start(d["y_d"][pr, :, jo, :, :], yo[:, :, :])


# ======================= host side =======================

def _prep_inputs(inputs):
    f32 = np.float32
    bf = ml_dtypes.bfloat16
    s1 = (inputs["bn1_g"] / np.sqrt(inputs["bn1_v"] + EPS)).astype(f32)
    Wfc1 = (inputs["w_fc1"] * s1[:, None]).astype(f32)
    b1 = ((inputs["b_fc1"] - inputs["bn1_m"]) * s1 + inputs["bn1_b"]).astype(f32)
    se = (inputs["bne_g"] / np.sqrt(inputs["bne_v"] + EPS)).astype(f32)
    W1 = inputs["w_ec"][:, :C]
    W2 = inputs["w_ec"][:, C:]
    WA = ((W1 - W2) * se[:, None]).astype(f32)
    WB = (W2 * se[:, None]).astype(f32)
    shift_e = ((inputs["b_ec"] - inputs["bne_m"]) * se + inputs["bne_b"]).astype(f32)
    s2 = (inputs["bn2_g"] / np.sqrt(inputs["bn2_v"] + EPS)).astype(f32)
    Wfc2 = (0.8 * inputs["w_fc2"] * s2[:, None]).astype(f32)
    wup = (0.2 * inputs["w_up"]).astype(f32)
    shift_out = (0.8 * ((inputs["b_fc2"] - inputs["bn2_m"]) * s2 + inputs["bn2_b"])
                 + 0.2 * inputs["b_up"]).astype(f32)

    def chunk_pj(a, nch):  # [nch*128, ...] -> [128, nch, ...]
        return np.ascontiguousarray(
            a.reshape(nch, 128, *a.shape[1:]).transpose(1, 0, *range(2, a.ndim + 1)))

    w = {
        "wfc1t": chunk_pj(Wfc1.T.copy(), CCH).astype(bf),       # [128,3,384]
        "bias1": chunk_pj(b1, CCH),                             # [128,3]
        "prom": chunk_pj(inputs["node_prompts"].astype(f32), CCH),
        "wdownt": chunk_pj(inputs["w_down"].T.astype(f32).copy(), CCH).astype(bf),
        "bdown": inputs["b_down"].astype(f32).reshape(R, 1),
        "gp": (0.2 * inputs["graph_prompt"]).astype(bf),        # [32,384]
        "wat": chunk_pj(WA.T.copy(), CCH).astype(bf),           # [128,3,768]
        "wbt": chunk_pj(WB.T.copy(), CCH).astype(bf),
        "shifte": chunk_pj(shift_e, C2CH),                      # [128,6]
        "wfc2t": chunk_pj(Wfc2.T.copy(), C2CH).astype(bf),      # [128,6,384]
        "wupt": wup.T.copy().astype(bf),                        # [32,384]
        "shifto": chunk_pj(shift_out, CCH),                     # [128,3]
    }
    w = {k: np.ascontiguousarray(v) for k, v in w.items()}
    return w


def _shard_x(x):
    # -> per-core [NPAIRS, 128, CCH, 2, HW] f32
    shards = []
    for c in range(NCORES):
        xs = x[c * SPC:(c + 1) * SPC].reshape(SPC, C, HW)
        xs = xs.reshape(NPAIRS, 2, CCH, 128, HW).transpose(0, 3, 2, 1, 4)
        shards.append(np.ascontiguousarray(xs.astype(np.float32)))
    return shards


def _unshard_y(results):
    out = np.empty((B, C, H, W), np.float32)
    for c in range(NCORES):
        y = results[c]["y_d"]  # [NPAIRS,128,CCH,2,HW]
        ys = y.transpose(0, 3, 2, 1, 4).reshape(SPC, C, H, W)
        out[c * SPC:(c + 1) * SPC] = ys
    return out


def get_program():
    if "nc" not in _CACHE:
        _CACHE["nc"] = _build_program()
    return _CACHE["nc"]


def run(inputs, trace=False, **kw):
    from concourse.bass_utils import run_bass_kernel_spmd
    nc = get_program()
    w = _prep_inputs(inputs)
    shards = _shard_x(np.asarray(inputs["x"], np.float32))
    import ml_dtypes as _md
    in_maps = [{**w, "x_d": shards[c],
                "xb_d": shards[c].astype(_md.bfloat16)} for c in range(NCORES)]
    res = run_bass_kernel_spmd(nc, in_maps, list(range(NCORES)), trace=trace, **kw)
    return _unshard_y(res.results), res


def kernel(**inputs):
    y, _ = run(inputs)
    return y


if __name__ == "__main__":
    get_program()
    print("program built OK")
